# revision 2
# baseline (speedup 1.0000x reference)
"""Trainium2 Bass kernel for nn_Basic_Aggregator (gnn_message_passing).

Math: out[b, i, :] = sum_j node_j[b, j, :]  (sum over node axis, broadcast
back to every row).  edge_ij is unused by the computation.

Sharding: data-parallel over batch B=16 across 8 cores (2 batches/core).
Each core reads its [2, 20000, 64] slab, reduces each batch to a [64]
vector, broadcasts it back to [20000, 64] and writes it out.  No
cross-core communication.

Layout: 20000 rows = 125 partitions x 160 rows, so a whole batch moves as
a single fully-contiguous DMA of [125, 10240] f32 (40960 B per
partition), with no remainder.
"""

import numpy as np

B, SIZE, D = 16, 20000, 64
N_CORES = 8
B_LOCAL = B // N_CORES  # 2
P = 125                 # partitions used; 125 * 160 = 20000 rows
NG = 160                # rows per partition
W = NG * D              # 10240 f32 per partition

_STATE = {}

# Results of the most recent device run (for test harness introspection).
LAST_RESULT = None


def _patch_drain_split():
    """The walrus build in this container accepts at most one sync-wait
    command per instruction; Tile's kernel-tail drain collects one wait per
    dangling proc (6 here) onto a single Drain.  Split it into a chain of
    single-wait drains on the same engine — identical semantics."""
    from concourse import tile
    import concourse.mybir as mybir
    from concourse.vector_clock import ScopedClock

    if getattr(tile.TileContext, "_ant_drain_split", False):
        return

    def _drain_and_barrier(self, tick_clock, wait_clock):
        drain_inst = self.nc.sync.drain()
        wait_clock.add_sem_waits(
            drain_inst.ins, ScopedClock({None: tick_clock.global_clock})
        )
        si = drain_inst.ins.sync_info
        if si is not None and si.on_wait and len(si.on_wait) > 1:
            waits = list(si.on_wait)
            upds = list(si.on_update or [])
            drain_inst.ins.sync_info = mybir.SyncInfo(
                on_wait=[waits[0]], on_update=[]
            )
            for i, w in enumerate(waits[1:]):
                extra = self.nc.sync.drain()
                extra.ins.sync_info = mybir.SyncInfo(
                    on_wait=[w],
                    on_update=upds if i == len(waits) - 2 else [],
                )

        self.nc.all_engine_barrier()
        assert self.sems is not None
        popped = self.nc._tile_sem_poison_stack.pop()
        assert popped is self._sem_poison
        self.nc.clear_and_free_semaphores(list(self.sems.allocated().values()))
        self.nc.all_engine_barrier()

    tile.TileContext._drain_and_barrier = _drain_and_barrier
    tile.TileContext._ant_drain_split = True


def _build_nc():
    import concourse.bass as bass
    import concourse.mybir as mybir
    from concourse import tile

    _patch_drain_split()

    f32 = mybir.dt.float32
    nc = bass.Bass()
    x = nc.declare_dram_parameter("x", [B_LOCAL, SIZE, D], f32, isOutput=False)
    y = nc.declare_dram_parameter("y", [B_LOCAL, SIZE, D], f32, isOutput=True)

    # load chunks in row-groups (sum = NG); last one small to shrink the
    # critical-path tail (its reduce is the last thing before the store
    # chain can begin).
    CHUNKS = [70, 70, 20]
    WREP = 5                    # store repeats; WIDE_W * WREP == W
    WIDE_W = W // WREP          # 2048 f32 per partition in the bcast tile

    with tile.TileContext(nc) as tc:
        with (
            tc.tile_pool(name="io", bufs=1) as io,
            tc.tile_pool(name="small", bufs=1) as small,
            tc.tile_pool(name="psum", bufs=2, space="PSUM") as psum,
        ):
            # all-ones [125,125]: one matmul both partition-reduces and
            # broadcasts: (ones.T @ part)[p, d] = sum_q part[q, d] for all p
            ones_sq = small.tile([P, P], f32, tag="ones_sq")
            nc.vector.memset(ones_sq[:], 1.0)

            # Phase 1: all loads up front (SP sequencer HWDGE), chunked.
            chunks = {}
            for b in range(B_LOCAL):
                xb = x[b].rearrange("(p w) d -> p (w d)", p=P)  # [125, 10240]
                o = 0
                for c, cg in enumerate(CHUNKS):
                    t = io.tile([P, cg * D], f32, tag=f"in{b}_{c}")
                    nc.sync.dma_start(out=t[:], in_=xb[:, o * D:(o + cg) * D])
                    chunks[b, c] = t
                    o += cg

            # Phase 2: per-chunk reduce, PE accumulate+broadcast, widen,
            # store (stores on ACT's HWDGE ring).
            for b in range(B_LOCAL):
                bc_psum = psum.tile([P, D], f32, tag="bc")
                for c, cg in enumerate(CHUNKS):
                    part = small.tile([P, D], f32, tag=f"part{b}_{c}")
                    view = chunks[b, c][:].rearrange("p (n d) -> p d n", d=D)
                    nc.vector.reduce_sum(part[:], view, axis=mybir.AxisListType.X)
                    nc.tensor.matmul(bc_psum[:], ones_sq[:], part[:],
                                     start=(c == 0), stop=(c == len(CHUNKS) - 1))

                # widen bc_psum [125,64] to [125, 2048] by doubling copies
                wide = io.tile([P, WIDE_W], f32, tag=f"wide{b}")
                nc.vector.tensor_copy(wide[:, 0:D], bc_psum[:])
                w = D
                while w < WIDE_W:
                    c = min(w, WIDE_W - w)
                    nc.vector.tensor_copy(wide[:, w:w + c], wide[:, 0:c])
                    w += c

                # store with a free-axis repeat: each partition's 160 rows
                # are 5 repeats of the 32-row pattern in `wide`.
                yb = y[b].rearrange("(p r w) d -> p r (w d)", p=P, r=WREP)
                src = wide[:].unsqueeze(1).broadcast_to([P, WREP, WIDE_W])
                nc.scalar.dma_start(out=yb, in_=src)

    return nc


def _get_nc():
    if "nc" not in _STATE:
        _STATE["nc"] = _build_nc()
    return _STATE["nc"]


def kernel(node_j, edge_ij=None):
    global LAST_RESULT
    import os
    from concourse.bass_utils import run_bass_kernel_spmd

    node_j = np.ascontiguousarray(np.asarray(node_j), dtype=np.float32)
    assert node_j.shape == (B, SIZE, D), node_j.shape

    nc = _get_nc()
    in_maps = [
        {"x": node_j[i * B_LOCAL:(i + 1) * B_LOCAL]} for i in range(N_CORES)
    ]
    kwargs = {}
    if os.environ.get("BASS_TRACE"):
        kwargs = {"trace": True}
    res = run_bass_kernel_spmd(nc, in_maps, core_ids=list(range(N_CORES)),
                               **kwargs)
    LAST_RESULT = res
    out = np.concatenate([r["y"] for r in res.results], axis=0)
    return out



# revision 3
# speedup vs baseline: 2.6685x; 2.6685x over previous
"""Trainium2 Bass kernel for nn_Basic_Aggregator (gnn_message_passing).

Math: out[b, i, :] = sum_j node_j[b, j, :]  (sum over the node axis,
broadcast back to every row).  edge_ij is unused by the computation.

Sharding: data-parallel over batch B=16 across 8 cores (2 batches/core).
Each core reads its [2, 20000, 64] slab, reduces each batch to a [64]
vector, broadcasts it back to [20000, 64] and writes it out.  No
cross-core communication.

Layout: 20000 rows split as [128 partitions x 156 rows] + a 32-row tail.
128 partitions is load-bearing: the HWDGE splits a DMA's partition range
into equal blocks across SDMA engines, and only a multiple of 16
engages all 16 engines (~26.5 GB/s each, ~420 GB/s aggregate).  125
partitions (the previous layout) ran on just 5 engines (~132 GB/s).

Per batch the main slab loads as two chunks of (96, 60) rows so the
row-sum can overlap the loads.  Row-sum = in-place halving adds on the
vector engine (contiguous access; faster than a strided reduce and the
final add lands in a fresh tile so consumers carry exactly one sync
wait).  Cross-partition sum + broadcast = one PE matmul with an
all-ones [128,128] matrix accumulated in PSUM (chunks + tail).  The
PSUM result is fanned out to a [128, 26*64] tile with a single
stride-0-broadcast ACT copy, and stored with a free-axis repeat.

Exactly 8 DMAs (tail load, 4 chunk loads, 2 main stores, 1 tail store):
Tile has 8 DMA-completion sem lanes (DMAHW0-7); a 9th DMA reuses a lane
and picks up a second sync wait, which this walrus build rejects
(single-sync-wait limit, same constraint _patch_drain_split handles for
the kernel-tail drain).
"""

import numpy as np

B, SIZE, D = 16, 20000, 64
N_CORES = 8
B_LOCAL = B // N_CORES  # 2
P = 128                 # partitions (multiple of 16 -> all 16 SDMA engines)
MR = 156                # main rows per partition; 128*156 = 19968
MAIN = P * MR           # 19968
TAIL = SIZE - MAIN      # 32
CHUNKS = (96, 60)       # row-chunks per partition (sum = MR)
WROW = 26               # rows per store descriptor; MR/WROW = 6 reps
R = MR // WROW

_STATE = {}

# Results of the most recent device run (for test harness introspection).
LAST_RESULT = None


def _patch_drain_split():
    """The walrus build in this container accepts at most one sync-wait
    command per instruction; Tile's kernel-tail drain collects one wait per
    dangling proc onto a single Drain.  Split it into a chain of
    single-wait drains on the same engine — identical semantics."""
    from concourse import tile
    import concourse.mybir as mybir
    from concourse.vector_clock import ScopedClock

    if getattr(tile.TileContext, "_ant_drain_split", False):
        return

    def _drain_and_barrier(self, tick_clock, wait_clock):
        drain_inst = self.nc.sync.drain()
        wait_clock.add_sem_waits(
            drain_inst.ins, ScopedClock({None: tick_clock.global_clock})
        )
        si = drain_inst.ins.sync_info
        if si is not None and si.on_wait and len(si.on_wait) > 1:
            waits = list(si.on_wait)
            upds = list(si.on_update or [])
            drain_inst.ins.sync_info = mybir.SyncInfo(
                on_wait=[waits[0]], on_update=[]
            )
            for i, w in enumerate(waits[1:]):
                extra = self.nc.sync.drain()
                extra.ins.sync_info = mybir.SyncInfo(
                    on_wait=[w],
                    on_update=upds if i == len(waits) - 2 else [],
                )

        self.nc.all_engine_barrier()
        assert self.sems is not None
        popped = self.nc._tile_sem_poison_stack.pop()
        assert popped is self._sem_poison
        self.nc.clear_and_free_semaphores(list(self.sems.allocated().values()))
        self.nc.all_engine_barrier()

    tile.TileContext._drain_and_barrier = _drain_and_barrier
    tile.TileContext._ant_drain_split = True


def _emit_rowsum(eng, t, rows, part):
    """Halving-add chain on tile t [P, rows*D]; the final add lands in
    `part` [P, D] so the downstream matmul sees a single-writer region
    (one sync wait)."""
    r = rows
    while r > 2:
        if r % 2 == 0:
            h = r // 2
            eng.tensor_add(t[:, 0:h * D], t[:, 0:h * D], t[:, h * D:r * D])
            r = h
        else:
            eng.tensor_add(t[:, 0:D], t[:, 0:D], t[:, (r - 1) * D:r * D])
            r -= 1
    if r == 2:
        eng.tensor_add(part[:], t[:, 0:D], t[:, D:2 * D])
    else:
        eng.tensor_copy(part[:], t[:, 0:D])


def _build_nc():
    import concourse.bass as bass
    import concourse.mybir as mybir
    from concourse import tile

    _patch_drain_split()

    f32 = mybir.dt.float32
    nc = bass.Bass()
    x = nc.declare_dram_parameter("x", [B_LOCAL, SIZE, D], f32, isOutput=False)
    y = nc.declare_dram_parameter("y", [B_LOCAL, SIZE, D], f32, isOutput=True)

    WIDE = WROW * D

    with tile.TileContext(nc) as tc:
        with (
            tc.tile_pool(name="io", bufs=1) as io,
            tc.tile_pool(name="small", bufs=1) as small,
            tc.tile_pool(name="psum", bufs=2, space="PSUM") as psum,
        ):
            ones = small.tile([P, P], f32, tag="ones")
            nc.vector.memset(ones[:], 1.0)

            # ---- loads on the SP HWDGE ring: tail first (it feeds the
            # last matmul of BOTH batches; issued first it never gates).
            tail_t = small.tile([TAIL, B_LOCAL * D], f32, tag="tail")
            tail_src = x[:, MAIN:SIZE, :].rearrange("b r d -> r b d")
            nc.sync.dma_start(
                out=tail_t[:].rearrange("r (b d) -> r b d", b=B_LOCAL),
                in_=tail_src)

            chunk_t = {}
            for b in range(B_LOCAL):
                xb = x[b][0:MAIN].rearrange("(p w) d -> p (w d)", p=P)
                o = 0
                for c, rc in enumerate(CHUNKS):
                    t = io.tile([P, rc * D], f32, tag=f"in{b}_{c}")
                    nc.sync.dma_start(out=t[:], in_=xb[:, o * D:(o + rc) * D])
                    chunk_t[b, c] = t
                    o += rc

            # ---- per-batch: rowsum chains -> PE accumulate -> fan out
            # -> store on the ACT HWDGE ring (engines round-robin between
            # the two rings at packet granularity, so stores overlap the
            # remaining loads).
            tail_out = small.tile([TAIL, B_LOCAL * D], f32, tag="tailout")
            for b in range(B_LOCAL):
                bc = psum.tile([P, D], f32, tag="bc")
                for c, rc in enumerate(CHUNKS):
                    t = chunk_t[b, c]
                    part = small.tile([P, D], f32, tag=f"part{b}_{c}")
                    _emit_rowsum(nc.vector, t, rc, part)
                    nc.tensor.matmul(bc[:], ones[:], part[:],
                                     start=(c == 0), stop=False)
                nc.tensor.matmul(bc[:], ones[0:TAIL, :],
                                 tail_t[:, b * D:(b + 1) * D],
                                 start=False, stop=True)

                wide = io.tile([P, WIDE], f32, tag=f"wide{b}")
                src = bc[:].unsqueeze(1).broadcast_to([P, WROW, D])
                nc.scalar.copy(wide[:].rearrange("p (r d) -> p r d", d=D),
                               src)
                nc.scalar.copy(tail_out[:, b * D:(b + 1) * D], bc[0:TAIL, :])

                yb = y[b][0:MAIN].rearrange("(p r w) d -> p r (w d)", p=P, r=R)
                nc.scalar.dma_start(
                    out=yb, in_=wide[:].unsqueeze(1).broadcast_to([P, R, WIDE]))

            tail_dst = y[:, MAIN:SIZE, :].rearrange("b r d -> r b d")
            nc.scalar.dma_start(
                out=tail_dst,
                in_=tail_out[:].rearrange("r (b d) -> r b d", b=B_LOCAL))

    return nc


def _get_nc():
    if "nc" not in _STATE:
        _STATE["nc"] = _build_nc()
    return _STATE["nc"]


def kernel(node_j, edge_ij=None):
    global LAST_RESULT
    import os
    from concourse.bass_utils import run_bass_kernel_spmd

    node_j = np.ascontiguousarray(np.asarray(node_j), dtype=np.float32)
    assert node_j.shape == (B, SIZE, D), node_j.shape

    nc = _get_nc()
    in_maps = [
        {"x": node_j[i * B_LOCAL:(i + 1) * B_LOCAL]} for i in range(N_CORES)
    ]
    kwargs = {}
    if os.environ.get("BASS_TRACE"):
        kwargs = {"trace": True}
    res = run_bass_kernel_spmd(nc, in_maps, core_ids=list(range(N_CORES)),
                               **kwargs)
    LAST_RESULT = res
    out = np.concatenate([r["y"] for r in res.results], axis=0)
    return out
